# revision 1
# baseline (speedup 1.0000x reference)
"""Multi-head attention (B=4, N=2048, D=1024, H=16) on 8 Trainium2 NeuronCores.

Sharding: tensor-parallel over heads — 2 heads per core. Each core computes
QKV^T for its heads from the (host-pre-transposed, bf16) full X^T, runs
attention, and produces a partial projection output (its 128 rows of w_proj,
bf16). The host sums the 8 partial outputs.

Layout (bf16 operands everywhere, fp32 PSUM accumulation):
  QKV^T [128=(h0|h1 dims), tok] = W_chunk.T @ XT_chunk     (bf16, N=512)
  S^T   [keys, q] = (K^T_h chunk).T @ Q^T_h                (64-contraction,
                     two heads packed in array row halves via tile_position)
  P^T   = exp(S^T)   one merged [128,1024] ScalarE activation per key block
                     (walrus: Activation is ACT-only; GpSimd cannot touch PSUM)
  V_nat [keys, dh]  via DMA-transpose (XBAR, off the PE entirely)
  O_nat [q, dh]     = (P^T chunk).T @ V_nat chunk          (64-col matmuls,
                     full 128-partition output + K=128 contraction)
  denom [q, 1]      = (P^T chunk).T @ ones                 (1-col matmuls into
                     the same PSUM accumulation bank)
  O_fin = O_nat * recip(denom)   (per-partition tensor_scalar on DVE)
  O^T   via PE transpose (bf16) -> ofin
  Y     [tok, 1024] = Ofin_chunk.T @ Wp   (128-contraction per core, proj
                     matmuls interleaved into the next q-block's kb loop)
"""

import sys
from contextlib import ExitStack

import numpy as np

for _p in ("/opt/trn_rl_repo", "/opt/pypackages"):
    if _p not in sys.path:
        sys.path.insert(0, _p)

B, N, D = 4, 2048, 1024
H, DH = 16, 64
NCORES = 8
HPC = H // NCORES  # heads per core = 2
P = 128
QB = 512  # query block / token block

_cache = {}


def _build(nbatch, ntok):
    """Build + compile the per-core bass program. Same program on all cores;
    per-core weights arrive as data."""
    import concourse.bacc as bacc
    import concourse.mybir as mybir
    import concourse.tile as tile
    from concourse.masks import make_identity

    f32 = mybir.dt.float32
    bf16 = mybir.dt.bfloat16
    Exp = mybir.ActivationFunctionType.Exp

    DCH = D // P          # 8 contraction chunks for the projections
    nqb = ntok // QB      # query blocks per batch
    nkb = ntok // P       # key blocks per batch
    QCH = QB // P         # query chunks (128) per query block = 4

    nc = bacc.Bacc("TRN2", target_bir_lowering=False, debug=False)

    xt_d = nc.dram_tensor("xt", [D, nbatch * ntok], bf16, kind="ExternalInput")
    wqkv_d = nc.dram_tensor("wqkv", [P, DCH * 3 * P], bf16, kind="ExternalInput")
    bq_d = nc.dram_tensor("bq", [P, 3], f32, kind="ExternalInput")
    wp_d = nc.dram_tensor("wp", [P, D], bf16, kind="ExternalInput")
    y_d = nc.dram_tensor("y", [nbatch * ntok, D], bf16, kind="ExternalOutput")

    with tile.TileContext(nc) as tc, ExitStack() as ctx:
        const = ctx.enter_context(tc.tile_pool(name="const", bufs=1))
        xt_pool = ctx.enter_context(tc.tile_pool(name="xt", bufs=16))
        qkvt_pool = ctx.enter_context(tc.tile_pool(name="qkvt", bufs=2))
        vn_pool = ctx.enter_context(tc.tile_pool(name="vn", bufs=2))
        pt_pool = ctx.enter_context(tc.tile_pool(name="pt", bufs=8))
        onat_pool = ctx.enter_context(tc.tile_pool(name="onat", bufs=2))
        ofin_pool = ctx.enter_context(tc.tile_pool(name="ofin", bufs=2))
        rc_pool = ctx.enter_context(tc.tile_pool(name="rc", bufs=2))
        yo_pool = ctx.enter_context(tc.tile_pool(name="yo", bufs=4))
        ps512 = ctx.enter_context(tc.tile_pool(name="ps512", bufs=2, space="PSUM"))
        pst = ctx.enter_context(tc.tile_pool(name="pst", bufs=2, space="PSUM"))
        pso = ctx.enter_context(tc.tile_pool(name="pso", bufs=1, space="PSUM"))
        psd = ctx.enter_context(tc.tile_pool(name="psd", bufs=1, space="PSUM"))

        # ---- constants ----
        # (w/xt chunk DMAs for batch 0 are issued first — wp/bq DMAs are
        # deferred into stage_a_xt_dma(0) since they're not needed until the
        # first projection / bias-add, long after startup)
        w_sb = const.tile([P, DCH * 3 * P], bf16, tag="w")
        wp_sb = const.tile([P, D], bf16, tag="wp")
        bq_sb = const.tile([P, 3], f32, tag="bq")
        ident = const.tile([P, P], bf16, tag="ident")
        make_identity(nc, ident)
        ones_col = const.tile([P, 1], bf16, tag="ones")
        nc.vector.memset(ones_col[:], 1.0)

        nA = nqb  # token groups (stage A) == query blocks (stage B) per batch

        def stage_a_xt_dma(bt):
            """DMA the full-batch X^T chunks (and, for bt==0, the weights).
            Batch 0 arrives in per-tokblock pieces so the first QKV matmul
            isn't gated on the full-batch transfer."""
            t0 = bt * ntok
            xts = []
            for d in range(DCH):
                t = xt_pool.tile([P, ntok], bf16, tag="xt", name=f"xt{d}")
                if bt != 0:
                    # two half-batch transfers: shorter exclusive holds on the
                    # DMA engines, so V-transposes interleave sooner
                    h = ntok // 2
                    nc.sync.dma_start(t[:, :h], xt_d[d * P : (d + 1) * P, t0 : t0 + h])
                    nc.sync.dma_start(
                        t[:, h:], xt_d[d * P : (d + 1) * P, t0 + h : t0 + ntok]
                    )
                xts.append(t)
            if bt == 0:
                # tokblock-major arrival order; the full W lands as ONE DMA
                # right after xt d0/d1 so the first QKV granule starts early
                # and later granules never outrun the arrival stream
                for tb in range(nqb):
                    for d in range(DCH):
                        nc.sync.dma_start(
                            xts[d][:, tb * QB : (tb + 1) * QB],
                            xt_d[d * P : (d + 1) * P, t0 + tb * QB : t0 + (tb + 1) * QB],
                        )
                        if tb == 0 and d == 1:
                            nc.sync.dma_start(
                                w_sb[:, : DCH * 192], wqkv_d[:, : DCH * 192]
                            )
                            nc.sync.dma_start(bq_sb[:], bq_d[:])
                        if tb == 0 and d == 3:
                            nc.sync.dma_start(
                                w_sb[:, DCH * 192 :], wqkv_d[:, DCH * 192 :]
                            )
                        if tb == min(1, nqb - 1) and d == DCH - 1:
                            nc.sync.dma_start(wp_sb[:], wp_d[:])
            return xts

        def make_qkv_granules(bt, tb, xts):
            """Split one token block's QKV work (3 cb × 8 accumulating
            matmuls + evictions) into 2-matmul granules that can interleave
            between other PE work. PSUM accumulation groups may pause across
            foreign matmuls — `start` zeroing is per-bank."""
            state = {}

            def granule(cb, dpair):
                qt, kt, vt = bt_tiles[bt]
                vn = vn_tiles[bt]
                dest = {0: qt, 1: kt, 2: vt}
                s0, s1 = tb * QB, (tb + 1) * QB
                if dpair == 0:
                    state[cb] = ps512.tile([P, QB], f32, tag="ps", name="psA")
                ps = state[cb]
                for d in range(2 * dpair, 2 * dpair + 2):
                    nc.tensor.matmul(
                        ps[:],
                        w_sb[:, d * 384 + cb * P : d * 384 + (cb + 1) * P],
                        xts[d][:, s0:s1],
                        start=(d == 0),
                        stop=(d == DCH - 1),
                        skip_group_check=True,
                    )
                if 2 * dpair + 2 == DCH:
                    # psum -> sbuf (bf16), adding the per-column qkv bias
                    nc.vector.tensor_scalar_add(
                        dest[cb][:, s0:s1], ps[:], bq_sb[:, cb : cb + 1]
                    )
                    if cb == 2:
                        # V natural layout via DMA XBAR transpose (off-PE)
                        for kbl in range(QB // P):
                            kb = tb * (QB // P) + kbl
                            nc.sync.dma_start(
                                vn[:, kb * P : (kb + 1) * P],
                                vt[:, kb * P : (kb + 1) * P],
                                transpose=True,
                            )

            # (cb0,cb1) pairs dp-major first, then cb2: only 2 accumulations
            # open at once (fits the 2-bank psum pool) and, for batch 0,
            # compute consumes xt chunk pairs no faster than DMA delivers
            order = [(cb, dp) for dp in range(DCH // 2) for cb in (0, 1)]
            order += [(2, dp) for dp in range(DCH // 2)]
            return [(lambda cb=cb, dp=dp: granule(cb, dp)) for cb, dp in order]

        def stage_a_tok(bt, tb, xts):
            """Standalone stage A token block (prologue slots only)."""
            for g in make_qkv_granules(bt, tb, xts):
                g()
            for _ in range(QCH):
                emit_otrp()

        # --- one-qblock-delayed transpose/projection pipeline state ---
        # queue entries: [bt, qb, onats, trp_idx, proj_j, yo_tile]
        pending = []
        epilogue = [False]
        NEB = D // QB
        Copy = mybir.ActivationFunctionType.Copy

        def emit_otrp():
            """Emit the next pending O^T transpose: PE transpose + DVE
            eviction (short chain; keeps the SP DMA queue free of
            compute-dependent waits)."""
            for ent in pending:
                bt, qb, onats, idx, _, _ = ent
                if idx >= QCH:
                    continue
                ofin = ofin_tiles[bt]
                trp = ps512.tile([P, P], bf16, tag="ps", name="trp")
                nc.tensor.transpose(trp[:], onats[idx], ident[:])
                nc.vector.tensor_copy(
                    ofin[:, qb * QB + idx * P : qb * QB + (idx + 1) * P], trp[:]
                )
                ent[3] += 1
                return

        def emit_proj():
            """Emit the next pending projection matmul (one [128 tok, 512]
            output block + yo eviction; y DMA after each tok-chunk pair).
            A tok-chunk's matmuls wait only for ITS ofin chunk's transpose."""
            if not pending:
                return
            ent = pending[0]
            bt, qb, onats, trp_idx, j, yo = ent
            tc_i, eb = j // NEB, j % NEB
            if tc_i >= trp_idx:  # this chunk's transpose not emitted yet
                return
            t0 = bt * ntok
            ofin = ofin_tiles[bt]
            tok = qb * QB + tc_i * P
            if eb == 0:
                yo = yo_pool.tile([P, D], bf16, tag="yo", name="yo")
                ent[5] = yo
            yp = ps512.tile([P, QB], f32, tag="ps", name="yp")
            nc.tensor.matmul(
                yp[:],
                ofin[:, tok : tok + P],
                wp_sb[:, eb * QB : (eb + 1) * QB],
                start=True,
                stop=True,
            )
            if epilogue[0] and eb == 1:
                nc.scalar.activation(yo[:, eb * QB : (eb + 1) * QB], yp[:], Copy)
            else:
                nc.vector.tensor_copy(yo[:, eb * QB : (eb + 1) * QB], yp[:])
            if epilogue[0]:
                # per-half y DMA: the first half transfers while the second
                # evicts, shortening the final drain chain
                nc.sync.dma_start(
                    y_d[t0 + tok : t0 + tok + P, eb * QB : (eb + 1) * QB],
                    yo[:, eb * QB : (eb + 1) * QB],
                )
            elif eb == NEB - 1:
                nc.sync.dma_start(y_d[t0 + tok : t0 + tok + P, :], yo[:])
            ent[4] += 1
            if ent[4] == QCH * NEB:
                pending.pop(0)

        def stage_b_qblock(bt, qb, a_work=None):
            """Attention for query block qb of batch bt, both heads.
            Projection matmuls of the previous qblock AND the next batch's
            QKV chains (a_work: list of thunks) are interleaved into the kb
            loop so the PE has queued work while the exp chains complete."""
            qt, kt, _ = bt_tiles[bt]
            vn = vn_tiles[bt]
            q0, q1 = qb * QB, (qb + 1) * QB
            oacc = pso.tile([P, HPC * QCH * DH], f32, tag="oacc", name="oacc")
            dn = psd.tile([P, HPC * QCH], f32, tag="dn", name="dn")

            def emit_o(ptile, kb):
                # start=True zeroes the whole 2KB PSUM bank, so only the FIRST
                # matmul of kb==0 in each bank (oacc / dn) carries start=True;
                # the other accumulation groups land on the just-zeroed bank.
                # denominator matmuls first on the final kb so the
                # reciprocal (which gates the norm->transpose chain) can
                # start before the last O matmuls finish
                order = ((1, 0) if kb == nkb - 1 else (0, 1))
                for which in order:
                    for h in range(HPC):
                        for qc in range(QCH):
                            lhs = ptile[:, h * QB + qc * P : h * QB + (qc + 1) * P]
                            g = h * QCH + qc
                            if which == 0:
                                nc.tensor.matmul(
                                    oacc[:, g * DH : (g + 1) * DH],
                                    lhs,
                                    vn[:, kb * P + h * DH : kb * P + (h + 1) * DH],
                                    start=(kb == 0 and g == 0),
                                    stop=(kb == nkb - 1),
                                    skip_group_check=True,
                                )
                            else:
                                nc.tensor.matmul(
                                    dn[:, g : g + 1],
                                    lhs,
                                    ones_col[:],
                                    start=(kb == 0 and g == 0),
                                    stop=(kb == nkb - 1),
                                    skip_group_check=True,
                                )

            # O matmuls are delayed two kb so the PE stream runs S(kb+1/kb+2)
            # while ACT/GpSimd compute exp(kb) — O(kb) never stalls on exp.
            a_work = list(a_work or [])
            odelay = 6 if bt == nbatch - 1 else 3
            pend = []
            for kb in range(nkb):
                # single S^T tile + ONE merged exp per kb: the ACT init
                # overhead (~370ns) is paid once per kb instead of per head
                st = pst.tile([P, HPC * QB], f32, tag="st", name="st")
                for h in range(HPC):
                    nc.tensor.matmul(
                        st[:, h * QB : (h + 1) * QB],
                        kt[h * DH : (h + 1) * DH, kb * P : (kb + 1) * P],
                        qt[h * DH : (h + 1) * DH, q0:q1],
                        start=True,
                        stop=True,
                        tile_position=(h * DH, 0),
                    )
                ptile = pt_pool.tile([P, HPC * QB], bf16, tag="pt", name="pt")
                nc.scalar.activation(ptile[:], st[:], Exp)
                pend.append((ptile, kb))
                if a_work:
                    a_work.pop(0)()  # one QKV granule per kb (12 of 16 kbs)
                if kb in (3, 7, 11, 14):
                    emit_otrp()
                if len(pend) >= 6:
                    emit_o(*pend.pop(0))
                if bt == nbatch - 1 and nbatch > 1:
                    emit_proj()  # last batch: drain the backlog as filler
                elif kb % 2 == 1 and kb < 13:
                    emit_proj()  # 6/slot; backlog feeds the last batch
            while a_work:
                a_work.pop(0)()
                emit_otrp()
            while pend:
                emit_o(*pend.pop(0))

            # normalize: per-partition (per-query) reciprocal of the denom
            rc = rc_pool.tile([P, HPC * QCH], f32, tag="rc", name="rc")
            nc.vector.reciprocal(rc[:], dn[:])
            # single broadcast-multiply normalizes all 8 groups at once:
            # onat[p, qc*128 + h*64 + j] = oacc[p, (h*4+qc)*64 + j] * rc[p, h*4+qc]
            onat = onat_pool.tile([P, QCH * P], bf16, tag="onat", name="onat")
            nc.vector.tensor_tensor(
                onat[:].rearrange("p (qc h j) -> p h qc j", qc=QCH, h=HPC),
                oacc[:].rearrange("p (h qc j) -> p h qc j", h=HPC, qc=QCH),
                rc[:]
                .rearrange("p (h qc) -> p h qc", h=HPC)
                .unsqueeze(3)
                .broadcast_to([P, HPC, QCH, DH]),
                mybir.AluOpType.mult,
            )
            onats = [onat[:, qc * P : (qc + 1) * P] for qc in range(QCH)]
            pending.append([bt, qb, onats, 0, 0, None])

        # ---- software-pipelined emission: stage A of batch b+1 interleaves
        # with stage B of batch b; O^T transposes and projection of each
        # qblock trail one slot behind its attention ----
        bt_tiles = {}
        xts_by_batch = {}
        vn_tiles = {}
        ofin_tiles = {}
        for step in range(nbatch + 1):
            if step < nbatch:
                qt = qkvt_pool.tile([P, ntok], bf16, tag="qt", name="qt")
                kt = qkvt_pool.tile([P, ntok], bf16, tag="kt", name="kt")
                vt = qkvt_pool.tile([P, ntok], bf16, tag="vt", name="vt")
                bt_tiles[step] = (qt, kt, vt)
                vn_tiles[step] = vn_pool.tile([P, nkb * P], bf16, tag="vn", name="vn")
                ofin_tiles[step] = ofin_pool.tile(
                    [P, ntok], bf16, tag="ofin", name="ofin"
                )
                if step == 0:
                    xts_by_batch[0] = stage_a_xt_dma(0)
            for i in range(nA):
                if i == 1 and step + 1 < nbatch:
                    # prefetch next batch's xt AFTER slot 0 so its transfers
                    # don't hold the DMA engines ahead of this step's V
                    # transposes and first-batch chunks
                    xts_by_batch[step + 1] = stage_a_xt_dma(step + 1)
                if step == 0:
                    stage_a_tok(step, i, xts_by_batch[0])
                elif step < nbatch:
                    a_work = make_qkv_granules(step, i, xts_by_batch[step])
                    stage_b_qblock(step - 1, i, a_work)
                else:
                    stage_b_qblock(step - 1, i)
        # epilogue: drain remaining transposes and projections, interleaved
        epilogue[0] = True
        while pending:
            emit_otrp()
            emit_otrp()
            emit_proj()
            emit_proj()
            emit_proj()
            emit_proj()

    nc.compile()
    return nc


def get_compiled(nbatch=B, ntok=N):
    key = (nbatch, ntok)
    if key not in _cache:
        _cache[key] = _build(nbatch, ntok)
    return _cache[key]


def make_core_inputs(x, w_qkv, b_qkv, w_proj):
    """Host-side sharding: returns (in_maps list for 8 cores)."""
    import ml_dtypes

    bf16 = ml_dtypes.bfloat16
    B_, N_, D_ = x.shape
    xt = np.ascontiguousarray(x.reshape(B_ * N_, D_).T).astype(bf16)
    in_maps = []
    for c in range(NCORES):
        heads = [HPC * c + i for i in range(HPC)]

        def wcols(s, scale=1.0):
            return np.concatenate(
                [w_qkv[:, s * D + h * DH : s * D + (h + 1) * DH] for h in heads], axis=1
            ) * scale

        def bcol(s, scale=1.0):
            return np.concatenate(
                [b_qkv[s * D + h * DH : s * D + (h + 1) * DH] for h in heads]
            ) * scale

        scale = float(DH) ** -0.5
        # [1024, 384] -> [128, 8*384] chunk layout (chunk d = rows d*128..)
        wq = np.concatenate([wcols(0, scale), wcols(1), wcols(2)], axis=1)
        wqkv_c = np.ascontiguousarray(
            np.concatenate([wq[d * P : (d + 1) * P, :] for d in range(D // P)], axis=1)
        ).astype(bf16)
        bq_c = np.stack([bcol(0, scale), bcol(1), bcol(2)], axis=1).astype(np.float32)
        bq_c = np.ascontiguousarray(bq_c)
        wp_c = np.ascontiguousarray(
            np.concatenate([w_proj[h * DH : (h + 1) * DH, :] for h in heads], axis=0)
        ).astype(bf16)
        in_maps.append({"xt": xt, "wqkv": wqkv_c, "bq": bq_c, "wp": wp_c})
    return in_maps


def kernel(x, w_qkv, b_qkv, w_proj, b_proj):
    x = np.asarray(x, dtype=np.float32)
    w_qkv = np.asarray(w_qkv, dtype=np.float32)
    b_qkv = np.asarray(b_qkv, dtype=np.float32)
    w_proj = np.asarray(w_proj, dtype=np.float32)
    b_proj = np.asarray(b_proj, dtype=np.float32)
    B_, N_, D_ = x.shape

    from concourse.bass_utils import run_bass_kernel_spmd

    nc = get_compiled(B_, N_)
    in_maps = make_core_inputs(x, w_qkv, b_qkv, w_proj)
    res = run_bass_kernel_spmd(nc, in_maps, core_ids=list(range(NCORES)))
    y = res.results[0]["y"].astype(np.float64)
    for r in res.results[1:]:
        y = y + r["y"].astype(np.float64)
    y = y + b_proj[None, :].astype(np.float64)
    return y.reshape(B_, N_, D_).astype(np.float32)

